# revision 7
# baseline (speedup 1.0000x reference)
"""Trainium2 Bass kernel for a 2-layer GAT (PyG GATConv semantics, eval mode).

v2 strategy (8 NeuronCores, SPMD, dst-sharded):
  - Core c owns dst nodes [c*NPC, (c+1)*NPC).  Phase A computes the node
    feature slab [h1 | a_src] (128 bf16 per row) plus an SBUF-resident a_dst
    table; slabs are AllGathered into a 100352-row bf16 table.
  - Edges (with self-loops) are grouped per (dst-block b, src-core chunk c)
    and sorted by local dst.  One dma_gather per (b, c) fetches the 256-B
    table rows by src; the real edge count rides in a runtime register with
    trailing -1 indices, so padding costs no DMA packets.
  - a_dst is never gathered: per (b, c) a transposed mask (node-on-partition)
    is built from host-precomputed per-node slot ranges with two DVE compare
    ops, and small TensorE matmuls broadcast a_dst to edge slots.
  - Per edge: w = exp(leaky_relu(a_src + a_dst)) (softmax shift invariance
    makes the segment max unnecessary).  S = [w | w*h] in bf16 is aggregated
    per dst block with 0/1 mask matmuls accumulating in PSUM f32.
  - Block epilogue: divide by the softmax denominator, bias, ELU, and the
    layer-2 augmented projection; AllGather slab2; run the same edge pipeline
    for layer 2 (single head) followed by log_softmax.
"""

import sys

if "/opt/trn_rl_repo" not in sys.path:
    sys.path.insert(0, "/opt/trn_rl_repo")

from dataclasses import dataclass

import numpy as np
import ml_dtypes

import concourse.bass as bass
import concourse.bacc as bacc
import concourse.tile as tile
import concourse.mybir as mybir
from concourse.masks import make_identity

F32 = mybir.dt.float32
BF16 = mybir.dt.bfloat16
I16 = mybir.dt.int16
I32 = mybir.dt.int32
BF = ml_dtypes.bfloat16

NEG_SLOPE = 0.2
SENT = 584.0  # dstv sentinel (exactly representable in bf16)


@dataclass(frozen=True)
class Cfg:
    N: int = 100000
    F: int = 128
    H1: int = 8
    C1: int = 8
    D2: int = 40
    NC: int = 8
    dbg: bool = False

    @property
    def D1(self):
        return self.H1 * self.C1  # 64

    @property
    def NPC(self):
        return self.N // self.NC  # 12500

    @property
    def NB(self):
        return (self.NPC + 127) // 128  # 98

    @property
    def NPCP(self):
        return self.NB * 128  # 12544


@dataclass(frozen=True)
class Plan:
    """Static slot structure shared by all cores (from max-over-core counts)."""
    tpbc: tuple  # [NB*NC] subtiles per (block, chunk), flattened b*NC+c
    maxsub: int  # max subtiles of any (b, c)

    @property
    def nreg(self):
        return len(self.tpbc)

    def slots(self, r):
        return self.tpbc[r] * 128

    @property
    def totsub(self):
        return sum(self.tpbc)

    @property
    def totslot(self):
        return self.totsub * 128


# ---------------------------------------------------------------- host side


def make_plan_and_maps(x, edge_index, cfg: Cfg):
    c = cfg
    src = np.concatenate([np.asarray(edge_index[0]), np.arange(c.N)]).astype(np.int64)
    dst = np.concatenate([np.asarray(edge_index[1]), np.arange(c.N)]).astype(np.int64)
    chunk = src // c.NPC          # src core
    lrow = src - chunk * c.NPC    # row within src core's slab

    per_core = []
    for core in range(c.NC):
        lo = core * c.NPC
        m = (dst >= lo) & (dst < lo + c.NPC)
        s_c, s_l, d_l = chunk[m], lrow[m], dst[m] - lo
        b = d_l >> 7
        key = b * c.NC + s_c
        order = np.lexsort((d_l, key))
        per_core.append((key[order], s_l[order], d_l[order]))

    NREG = c.NB * c.NC
    cnts = np.zeros((c.NC, NREG), np.int64)
    for core in range(c.NC):
        cnts[core] = np.bincount(per_core[core][0], minlength=NREG)
    tpbc = np.maximum(1, np.ceil(cnts.max(axis=0) / 128.0)).astype(np.int64)
    plan = Plan(tpbc=tuple(int(t) for t in tpbc), maxsub=int(tpbc.max()))

    off_slot = np.zeros(NREG + 1, np.int64)
    off_slot[1:] = np.cumsum(tpbc * 128)
    off_sub = np.zeros(NREG + 1, np.int64)
    off_sub[1:] = np.cumsum(tpbc)
    TOTSLOT, TOTSUB = int(off_slot[-1]), int(off_sub[-1])

    iota_slot = np.tile(np.arange(plan.maxsub * 128, dtype=np.float32), (128, 1))
    iota_row = np.tile(np.arange(128, dtype=BF), (128, 1))
    blockmask = np.zeros((c.D1, c.H1), np.float32)
    for h in range(c.H1):
        blockmask[h * c.C1 : (h + 1) * c.C1, h] = 1.0

    in_maps = []
    for core in range(c.NC):
        key, s_l, d_l = per_core[core]
        starts = np.zeros(NREG, np.int64)
        cc = np.bincount(key, minlength=NREG)
        starts[1:] = np.cumsum(cc)[:-1]

        gidx = np.full(TOTSLOT, -1, np.int16)
        dstv = np.full(TOTSLOT, SENT, BF)
        counts = np.zeros(NREG, np.int32)
        se = np.zeros((128, NREG * 2), np.float32)
        for r in range(NREG):
            n = int(cc[r])
            st = int(starts[r])
            o = int(off_slot[r])
            counts[r] = max(n, 1)
            if n == 0:
                gidx[o] = 0  # keep >=1 valid idx per call
                continue
            gidx[o : o + n] = s_l[st : st + n]
            dl = d_l[st : st + n]
            dstv[o : o + n] = (dl & 127).astype(BF)
            # per-node slot ranges within this (b, c) for the transposed mask
            nib = (dl & 127).astype(np.int64)
            deg = np.bincount(nib, minlength=128)
            e_ = np.cumsum(deg)
            s_ = e_ - deg
            se[:, 2 * r] = s_.astype(np.float32)
            se[:, 2 * r + 1] = e_.astype(np.float32)

        # wrap gidx per region: [slots] -> [128, slots/16]
        gw = np.zeros((128, TOTSLOT // 16), np.int16)
        for r in range(NREG):
            o, s = int(off_slot[r]), int(plan.slots(r))
            w = gidx[o : o + s].reshape(s // 16, 16).T  # [16, s/16]
            gw[:, o // 16 : (o + s) // 16] = np.tile(w, (8, 1))

        dstv2 = dstv.reshape(TOTSUB, 128).T.copy()  # [128, TOTSUB]

        xs = np.zeros((c.NPCP, c.F), np.float32)
        xs[: c.NPC] = np.asarray(x)[core * c.NPC : (core + 1) * c.NPC]

        in_maps.append(
            {
                "x_slice": xs,
                "gidx": gw,
                "dstv": dstv2,
                "se": se,
                "counts": counts[None, :],
                "iota_slot": iota_slot,
                "iota_row": iota_row,
                "blockmask": blockmask,
            }
        )
    return plan, in_maps


# -------------------------------------------------------------- device side


def build(nc, cfg: Cfg, plan: Plan, repeats: int = 1):
    c = cfg
    D1, D2, H1 = c.D1, c.D2, c.H1
    TROWS = c.NC * c.NPCP
    NREG = c.NB * c.NC
    off_slot = np.zeros(NREG + 1, np.int64)
    off_slot[1:] = np.cumsum(np.asarray(plan.tpbc) * 128)
    off_sub = np.zeros(NREG + 1, np.int64)
    off_sub[1:] = np.cumsum(np.asarray(plan.tpbc))

    x_slice = nc.dram_tensor("x_slice", [c.NPCP, c.F], F32, kind="ExternalInput")
    W1 = nc.dram_tensor("W1", [c.F, D1], F32, kind="ExternalInput")
    att_src1 = nc.dram_tensor("att_src1", [H1, c.C1], F32, kind="ExternalInput")
    att_dst1 = nc.dram_tensor("att_dst1", [H1, c.C1], F32, kind="ExternalInput")
    b1 = nc.dram_tensor("b1", [D1], F32, kind="ExternalInput")
    W2 = nc.dram_tensor("W2", [D1, D2], F32, kind="ExternalInput")
    att_src2 = nc.dram_tensor("att_src2", [1, D2], F32, kind="ExternalInput")
    att_dst2 = nc.dram_tensor("att_dst2", [1, D2], F32, kind="ExternalInput")
    b2 = nc.dram_tensor("b2", [D2], F32, kind="ExternalInput")
    gidx_t = nc.dram_tensor("gidx", [128, plan.totslot // 16], I16, kind="ExternalInput")
    dstv_t = nc.dram_tensor("dstv", [128, plan.totsub], BF16, kind="ExternalInput")
    se_t = nc.dram_tensor("se", [128, NREG * 2], F32, kind="ExternalInput")
    counts_t = nc.dram_tensor("counts", [1, NREG], I32, kind="ExternalInput")
    iota_slot_t = nc.dram_tensor("iota_slot", [128, plan.maxsub * 128], F32, kind="ExternalInput")
    iota_row_t = nc.dram_tensor("iota_row", [128, 128], BF16, kind="ExternalInput")
    bmask_t = nc.dram_tensor("blockmask", [D1, H1], F32, kind="ExternalInput")
    out_t = nc.dram_tensor("out", [c.NPCP, D2], F32, kind="ExternalOutput")

    with tile.TileContext(nc) as tc:
        with (
            tc.tile_pool(name="dram", bufs=1, space="DRAM") as dram,
            tc.tile_pool(name="cst", bufs=1) as cst,
            tc.tile_pool(name="wk", bufs=2) as wk,
            tc.tile_pool(name="gp", bufs=8) as gp,
            tc.tile_pool(name="ps", bufs=1, space="PSUM") as ps,
        ):
            # ---- constants
            ident = cst.tile([128, 128], F32)
            make_identity(nc, ident[:])
            iota = cst.tile([128, 128], BF16)
            nc.sync.dma_start(out=iota[:], in_=iota_row_t.ap())
            iosl = cst.tile([128, plan.maxsub * 128], F32)
            nc.sync.dma_start(out=iosl[:], in_=iota_slot_t.ap())
            dstv = cst.tile([128, plan.totsub], BF16)
            nc.sync.dma_start(out=dstv[:], in_=dstv_t.ap())
            se = cst.tile([128, NREG * 2], F32)
            nc.sync.dma_start(out=se[:], in_=se_t.ap())
            counts = cst.tile([1, NREG], I32)
            nc.sync.dma_start(out=counts[:], in_=counts_t.ap())

            w1sb = cst.tile([c.F, D1], F32)
            nc.sync.dma_start(out=w1sb[:], in_=W1.ap())
            w1T_ps = ps.tile([D1, c.F], F32, tag="pT", bufs=2)
            nc.tensor.transpose(out=w1T_ps[:], in_=w1sb[:], identity=ident[:])
            w1T = cst.tile([D1, c.F], F32)
            nc.vector.tensor_copy(out=w1T[:], in_=w1T_ps[:])

            bmask = cst.tile([D1, H1], F32)
            nc.sync.dma_start(out=bmask[:], in_=bmask_t.ap())
            atts_c = cst.tile([D1, 2], F32)
            nc.sync.dma_start(
                out=atts_c[:, 0:1], in_=att_src1.ap().rearrange("h c -> (h c)")[:, None]
            )
            nc.sync.dma_start(
                out=atts_c[:, 1:2], in_=att_dst1.ap().rearrange("h c -> (h c)")[:, None]
            )
            ablk = cst.tile([D1, 2 * H1], F32)
            nc.vector.tensor_tensor(
                out=ablk[:, 0:H1],
                in0=atts_c[:, 0:1].to_broadcast([D1, H1]),
                in1=bmask[:],
                op=mybir.AluOpType.mult,
            )
            nc.vector.tensor_tensor(
                out=ablk[:, H1 : 2 * H1],
                in0=atts_c[:, 1:2].to_broadcast([D1, H1]),
                in1=bmask[:],
                op=mybir.AluOpType.mult,
            )
            acols_ps = ps.tile([c.F, 2 * H1], F32, tag="pT", bufs=2)
            nc.tensor.matmul(out=acols_ps[:], lhsT=w1T[:], rhs=ablk[:], start=True, stop=True)
            w1aug = cst.tile([c.F, D1 + 2 * H1], F32)
            nc.vector.tensor_copy(out=w1aug[:, 0:D1], in_=w1sb[:])
            nc.vector.tensor_copy(out=w1aug[:, D1 : D1 + 2 * H1], in_=acols_ps[:])

            w2sb = cst.tile([D1, D2], F32)
            nc.sync.dma_start(out=w2sb[:], in_=W2.ap())
            w2T_ps = ps.tile([D2, D1], F32, tag="pT", bufs=2)
            nc.tensor.transpose(out=w2T_ps[:], in_=w2sb[:], identity=ident[0:D1, 0:D1])
            w2T = cst.tile([D2, D1], F32)
            nc.vector.tensor_copy(out=w2T[:], in_=w2T_ps[:])
            att2 = cst.tile([D2, 2], F32)
            nc.sync.dma_start(out=att2[:, 0:1], in_=att_src2.ap().rearrange("o c -> (o c)")[:, None])
            nc.sync.dma_start(out=att2[:, 1:2], in_=att_dst2.ap().rearrange("o c -> (o c)")[:, None])
            v2_ps = ps.tile([D1, 2], F32, tag="pT", bufs=2)
            nc.tensor.matmul(out=v2_ps[:], lhsT=w2T[:], rhs=att2[:], start=True, stop=True)
            w2aug = cst.tile([D1, D2 + 2], F32)
            nc.vector.tensor_copy(out=w2aug[:, 0:D2], in_=w2sb[:])
            nc.vector.tensor_copy(out=w2aug[:, D2 : D2 + 2], in_=v2_ps[:])

            ones1 = cst.tile([1, 128], F32)
            nc.vector.memset(ones1[:], 1.0)
            b1row = cst.tile([1, D1], F32)
            nc.sync.dma_start(out=b1row[:], in_=b1.ap()[None, :])
            b1rep_ps = ps.tile([128, D1], F32, tag="pT", bufs=2)
            nc.tensor.matmul(out=b1rep_ps[:], lhsT=ones1[:], rhs=b1row[:], start=True, stop=True)
            b1rep = cst.tile([128, D1], F32)
            nc.vector.tensor_copy(out=b1rep[:], in_=b1rep_ps[:])
            b2row = cst.tile([1, D2], F32)
            nc.sync.dma_start(out=b2row[:], in_=b2.ap()[None, :])
            b2rep_ps = ps.tile([128, D2], F32, tag="pT", bufs=2)
            nc.tensor.matmul(out=b2rep_ps[:], lhsT=ones1[:], rhs=b2row[:], start=True, stop=True)
            b2rep = cst.tile([128, D2], F32)
            nc.vector.tensor_copy(out=b2rep[:], in_=b2rep_ps[:])

            # a_dst tables (SBUF resident, rewritten per rep)
            adall1 = cst.tile([128, c.NB * H1], BF16)
            adall2 = cst.tile([128, c.NB], BF16)

            # prime gather bufs so -1 tail slots hold finite bf16 data forever
            for _ in range(8):
                gpr = gp.tile([128, plan.maxsub * 128], BF16, tag="g")
                nc.vector.memset(gpr[:], 0.0)

            for _rep in range(repeats):
                slab1 = dram.tile([c.NPCP, 128], BF16, tag=f"slab1_{_rep}")
                table1 = dram.tile([TROWS, 128], BF16, addr_space="Shared", tag=f"table1_{_rep}")
                slab2 = dram.tile([c.NPCP, 128], BF16, tag=f"slab2_{_rep}")
                table2 = dram.tile([TROWS, 128], BF16, addr_space="Shared", tag=f"table2_{_rep}")

                # ---- phase A: node features, attention columns
                for t in range(c.NB):
                    xt = wk.tile([128, c.F], F32, tag="xt")
                    nc.sync.dma_start(out=xt[:], in_=x_slice.ap()[t * 128 : (t + 1) * 128, :])
                    xT_ps = ps.tile([c.F, 128], F32, tag="pT", bufs=2)
                    nc.tensor.transpose(out=xT_ps[:], in_=xt[:], identity=ident[:])
                    xTs = wk.tile([c.F, 128], F32, tag="xTs")
                    nc.vector.tensor_copy(out=xTs[:], in_=xT_ps[:])
                    h_ps = ps.tile([128, D1 + 2 * H1], F32, tag="psA", bufs=2)
                    nc.tensor.matmul(out=h_ps[:], lhsT=xTs[:], rhs=w1aug[:], start=True, stop=True)
                    s1o = wk.tile([128, 128], BF16, tag="s1o")
                    nc.vector.memset(s1o[:, D1 + H1 : 128], 0.0)
                    nc.vector.tensor_copy(out=s1o[:, 0 : D1 + H1], in_=h_ps[:, 0 : D1 + H1])
                    nc.sync.dma_start(out=slab1[t * 128 : (t + 1) * 128, :], in_=s1o[:])
                    nc.vector.tensor_copy(
                        out=adall1[:, t * H1 : (t + 1) * H1],
                        in_=h_ps[:, D1 + H1 : D1 + 2 * H1],
                    )

                nc.gpsimd.collective_compute(
                    "AllGather",
                    mybir.AluOpType.bypass,
                    replica_groups=[list(range(c.NC))],
                    ins=[slab1.opt()],
                    outs=[table1.opt()],
                )

                def edge_layer(layer):
                    creg = nc.gpsimd.alloc_register(f"cnt_reg_{_rep}_{layer}")
                    NH = H1 if layer == 1 else 1
                    DV = D1 if layer == 1 else D2
                    SW = NH + DV
                    table = table1 if layer == 1 else table2
                    adall = adall1 if layer == 1 else adall2
                    for b in range(c.NB):
                        agg = ps.tile([128, SW], F32, tag="agg", bufs=3)
                        r0 = b * c.NC
                        # per-block slice of wrapped gather indices
                        gcols0 = int(off_slot[r0]) // 16
                        gcolsN = int(off_slot[r0 + c.NC]) // 16 - gcols0
                        gix = wk.tile([128, gcolsN], I16, tag="gix", bufs=3)
                        nc.sync.dma_start(out=gix[:], in_=gidx_t.ap()[:, gcols0 : gcols0 + gcolsN])
                        subdone = 0
                        for ci in range(c.NC):
                            r = r0 + ci
                            T = plan.tpbc[r]
                            S_r = T * 128
                            nc.gpsimd.reg_load(creg, counts[0:1, r : r + 1])
                            g = gp.tile([128, plan.maxsub * 128], BF16, tag="g")
                            g3 = g[:].rearrange("p (t e) -> p t e", e=128)
                            nc.gpsimd.dma_gather(
                                out_ap=g3[:, 0:T, :],
                                in_ap=table[:][ci * c.NPCP : (ci + 1) * c.NPCP, :],
                                idxs_ap=gix[:, (int(off_slot[r]) // 16 - gcols0) : (int(off_slot[r + 1]) // 16 - gcols0)],
                                num_idxs=S_r,
                                num_idxs_reg=creg,
                                elem_size=128,
                                single_packet=False,
                                queue_num=r % 4,
                            )
                            # transposed mask from per-node slot ranges
                            mT = wk.tile([128, plan.maxsub * 128], BF16, tag="mT", bufs=3)
                            nc.vector.tensor_scalar(
                                out=mT[:, 0:S_r],
                                in0=iosl[:, 0:S_r],
                                scalar1=se[:, 2 * r : 2 * r + 1],
                                scalar2=None,
                                op0=mybir.AluOpType.is_ge,
                            )
                            t2 = wk.tile([128, plan.maxsub * 128], BF16, tag="t2", bufs=2)
                            nc.vector.tensor_scalar(
                                out=t2[:, 0:S_r],
                                in0=iosl[:, 0:S_r],
                                scalar1=se[:, 2 * r + 1 : 2 * r + 2],
                                scalar2=None,
                                op0=mybir.AluOpType.is_lt,
                            )
                            nc.vector.tensor_tensor(
                                out=mT[:, 0:S_r], in0=mT[:, 0:S_r], in1=t2[:, 0:S_r],
                                op=mybir.AluOpType.mult,
                            )
                            # a_dst broadcast to edge slots: ed[e, h]
                            edps = ps.tile([128, plan.maxsub * NH], F32, tag="psA", bufs=2)
                            for j in range(T):
                                nc.tensor.matmul(
                                    out=edps[:, j * NH : (j + 1) * NH],
                                    lhsT=mT[:, j * 128 : (j + 1) * 128],
                                    rhs=adall[:, b * NH : (b + 1) * NH],
                                    start=True,
                                    stop=True,
                                )
                            ew = wk.tile([128, plan.maxsub * NH], BF16, tag="ew", bufs=3)
                            nc.scalar.copy(out=ew[:, : T * NH], in_=edps[:, : T * NH])
                            # e = a_src + a_dst ; w = exp(lrelu(e))
                            nc.vector.tensor_tensor(
                                out=ew[:, : T * NH].rearrange("p (t h) -> p t h", h=NH),
                                in0=g3[:, 0:T, DV : DV + NH],
                                in1=ew[:, : T * NH].rearrange("p (t h) -> p t h", h=NH),
                                op=mybir.AluOpType.add,
                            )
                            nc.vector.scalar_tensor_tensor(
                                out=ew[:, : T * NH],
                                in0=ew[:, : T * NH],
                                scalar=NEG_SLOPE,
                                in1=ew[:, : T * NH],
                                op0=mybir.AluOpType.mult,
                                op1=mybir.AluOpType.max,
                            )
                            nc.scalar.activation(
                                out=ew[:, : T * NH], in_=ew[:, : T * NH],
                                func=mybir.ActivationFunctionType.Exp,
                            )
                            # mask (edge-on-partition) and S
                            sub0 = int(off_sub[r])
                            mask = wk.tile([128, plan.maxsub * 128], BF16, tag="mask", bufs=3)
                            nc.vector.tensor_tensor(
                                out=mask[:, : T * 128].rearrange("p (t n) -> p t n", n=128),
                                in0=dstv[:, sub0 : sub0 + T][:, :, None].to_broadcast(
                                    [128, T, 128]
                                ),
                                in1=iota[:][:, None, :].to_broadcast([128, T, 128]),
                                op=mybir.AluOpType.is_equal,
                            )
                            S = wk.tile([128, plan.maxsub * SW], BF16, tag="S", bufs=3)
                            S3 = S[:, : T * SW].rearrange("p (t e) -> p t e", e=SW)
                            nc.vector.tensor_copy(
                                out=S3[:, :, 0:NH],
                                in_=ew[:, : T * NH].rearrange("p (t h) -> p t h", h=NH),
                            )
                            if layer == 1:
                                nc.vector.tensor_tensor(
                                    out=S3[:, :, NH:SW].rearrange("p t (h ch) -> p t h ch", h=NH),
                                    in0=g3[:, 0:T, 0:DV].rearrange("p t (h ch) -> p t h ch", h=NH),
                                    in1=ew[:, : T * NH]
                                    .rearrange("p (t h) -> p t h", h=NH)[:, :, :, None]
                                    .to_broadcast([128, T, NH, c.C1]),
                                    op=mybir.AluOpType.mult,
                                )
                            else:
                                nc.vector.tensor_tensor(
                                    out=S3[:, :, NH:SW],
                                    in0=g3[:, 0:T, 0:DV],
                                    in1=ew[:, : T * NH][:, :, None].to_broadcast([128, T, DV]),
                                    op=mybir.AluOpType.mult,
                                )
                            for j in range(T):
                                nc.tensor.matmul(
                                    out=agg[:],
                                    lhsT=mask[:, j * 128 : (j + 1) * 128],
                                    rhs=S3[:, j, :],
                                    start=(ci == 0 and j == 0),
                                    stop=(ci == c.NC - 1 and j == T - 1),
                                )
                            subdone += T

                        # ---- block epilogue
                        ssafe = wk.tile([128, NH], F32, tag="ssafe")
                        nc.vector.tensor_scalar_max(out=ssafe[:], in0=agg[:, 0:NH], scalar1=1e-16)
                        rec = wk.tile([128, NH], F32, tag="rec")
                        nc.vector.reciprocal(out=rec[:], in_=ssafe[:])
                        o1 = wk.tile([128, DV], F32, tag="o1")
                        if layer == 1:
                            nc.vector.tensor_tensor(
                                out=o1[:].rearrange("p (h ch) -> p h ch", h=NH),
                                in0=agg[:, NH:SW].rearrange("p (h ch) -> p h ch", h=NH),
                                in1=rec[:][:, :, None].to_broadcast([128, NH, c.C1]),
                                op=mybir.AluOpType.mult,
                            )
                            nc.vector.tensor_add(out=o1[:], in0=o1[:], in1=b1rep[:])
                            neg = wk.tile([128, DV], F32, tag="neg")
                            nc.vector.tensor_scalar_min(out=neg[:], in0=o1[:], scalar1=0.0)
                            nc.scalar.activation(
                                out=neg[:], in_=neg[:], func=mybir.ActivationFunctionType.Exp
                            )
                            pos = wk.tile([128, DV], F32, tag="pos")
                            nc.vector.tensor_scalar_max(out=pos[:], in0=o1[:], scalar1=0.0)
                            elu = wk.tile([128, DV], F32, tag="elu")
                            nc.vector.scalar_tensor_tensor(
                                out=elu[:], in0=neg[:], scalar=-1.0, in1=pos[:],
                                op0=mybir.AluOpType.add, op1=mybir.AluOpType.add,
                            )
                            eT_ps = ps.tile([DV, 128], F32, tag="pT", bufs=2)
                            nc.tensor.transpose(out=eT_ps[:], in_=elu[:], identity=ident[:])
                            eT = wk.tile([DV, 128], F32, tag="eT")
                            nc.vector.tensor_copy(out=eT[:], in_=eT_ps[:])
                            h2_ps = ps.tile([128, D2 + 2], F32, tag="psA", bufs=2)
                            nc.tensor.matmul(out=h2_ps[:], lhsT=eT[:], rhs=w2aug[:], start=True, stop=True)
                            s2o = wk.tile([128, 128], BF16, tag="s2o")
                            nc.vector.memset(s2o[:, D2 + 1 : 128], 0.0)
                            nc.vector.tensor_copy(out=s2o[:, 0 : D2 + 1], in_=h2_ps[:, 0 : D2 + 1])
                            nc.sync.dma_start(out=slab2[b * 128 : (b + 1) * 128, :], in_=s2o[:])
                            nc.vector.tensor_copy(
                                out=adall2[:, b : b + 1], in_=h2_ps[:, D2 + 1 : D2 + 2]
                            )
                        else:
                            nc.vector.tensor_tensor(
                                out=o1[:],
                                in0=agg[:, NH:SW],
                                in1=rec[:].to_broadcast([128, DV]),
                                op=mybir.AluOpType.mult,
                            )
                            nc.vector.tensor_add(out=o1[:], in0=o1[:], in1=b2rep[:])
                            mx = wk.tile([128, 1], F32, tag="mx")
                            nc.vector.tensor_reduce(
                                out=mx[:], in_=o1[:], axis=mybir.AxisListType.X,
                                op=mybir.AluOpType.max,
                            )
                            xm = wk.tile([128, DV], F32, tag="xm")
                            nc.vector.tensor_tensor(
                                out=xm[:], in0=o1[:], in1=mx[:].to_broadcast([128, DV]),
                                op=mybir.AluOpType.subtract,
                            )
                            ex = wk.tile([128, DV], F32, tag="ex")
                            sm = wk.tile([128, 1], F32, tag="sm")
                            nc.scalar.activation(
                                out=ex[:], in_=xm[:], func=mybir.ActivationFunctionType.Exp
                            )
                            nc.vector.tensor_reduce(
                                out=sm[:], in_=ex[:], axis=mybir.AxisListType.X,
                                op=mybir.AluOpType.add,
                            )
                            lg = wk.tile([128, 1], F32, tag="lg")
                            nc.scalar.activation(
                                out=lg[:], in_=sm[:], func=mybir.ActivationFunctionType.Ln
                            )
                            oo = wk.tile([128, DV], F32, tag="oo")
                            nc.vector.tensor_tensor(
                                out=oo[:], in0=xm[:], in1=lg[:].to_broadcast([128, DV]),
                                op=mybir.AluOpType.subtract,
                            )
                            nc.sync.dma_start(out=out_t.ap()[b * 128 : (b + 1) * 128, :], in_=oo[:])

                edge_layer(1)
                nc.gpsimd.collective_compute(
                    "AllGather",
                    mybir.AluOpType.bypass,
                    replica_groups=[list(range(c.NC))],
                    ins=[slab2.opt()],
                    outs=[table2.opt()],
                )
                edge_layer(2)


# ------------------------------------------------------------------ driver


def make_runner(nc, n_cores=8):
    import jax
    from jax.sharding import Mesh, PartitionSpec
    from jax.experimental.shard_map import shard_map
    from concourse.bass2jax import (
        _bass_exec_p,
        install_neuronx_cc_hook,
        partition_id_tensor,
    )

    install_neuronx_cc_hook()
    partition_name = nc.partition_id_tensor.name if nc.partition_id_tensor else None

    in_names, out_names, out_avals, zero_outs = [], [], [], []
    for alloc in nc.m.functions[0].allocations:
        if not isinstance(alloc, mybir.MemoryLocationSet):
            continue
        name = alloc.memorylocations[0].name
        if alloc.kind == "ExternalInput":
            if name != partition_name:
                in_names.append(name)
        elif alloc.kind == "ExternalOutput":
            shape = tuple(alloc.tensor_shape)
            dtype = mybir.dt.np(alloc.dtype)
            out_names.append(name)
            out_avals.append(jax.core.ShapedArray(shape, dtype))
            zero_outs.append(np.zeros(shape, dtype))
    n_params = len(in_names)
    n_outs = len(out_avals)
    all_in_names = list(in_names) + list(out_names)
    if partition_name is not None:
        all_in_names.append(partition_name)

    donate = tuple(range(n_params, n_params + n_outs))

    def _body(*args):
        operands = list(args)
        if partition_name is not None:
            operands.append(partition_id_tensor())
        outs = _bass_exec_p.bind(
            *operands,
            out_avals=tuple(out_avals),
            in_names=tuple(all_in_names),
            out_names=tuple(out_names),
            lowering_input_output_aliases=(),
            sim_require_finite=False,
            sim_require_nnan=False,
            nc=nc,
        )
        return tuple(outs)

    devices = jax.devices()[:n_cores]
    mesh = Mesh(np.asarray(devices), ("core",))
    in_specs = (PartitionSpec("core"),) * (n_params + n_outs)
    out_specs = (PartitionSpec("core"),) * len(out_names)
    sharded = jax.jit(
        shard_map(_body, mesh=mesh, in_specs=in_specs, out_specs=out_specs, check_rep=False),
        donate_argnums=donate,
        keep_unused=True,
    )

    def run(in_maps):
        per_core = [[np.asarray(m[name]) for name in in_names] for m in in_maps]
        concat_in = [
            np.concatenate([per_core[cc][i] for cc in range(n_cores)], axis=0)
            for i in range(n_params)
        ]
        concat_zeros = [
            np.zeros((n_cores * z.shape[0], *z.shape[1:]), z.dtype) for z in zero_outs
        ]
        out_arrs = sharded(*concat_in, *concat_zeros)
        jax.block_until_ready(out_arrs)
        return [
            {
                name: np.asarray(out_arrs[i]).reshape(n_cores, *out_avals[i].shape)[cc]
                for i, name in enumerate(out_names)
            }
            for cc in range(n_cores)
        ]

    return run


_CACHE = {}


def _get_runner(cfg: Cfg, plan: Plan, repeats: int = 1):
    key = (cfg, plan, repeats)
    if key in _CACHE:
        return _CACHE[key]
    nc = bacc.Bacc(
        "TRN2",
        target_bir_lowering=False,
        debug=False,
        num_devices=cfg.NC,
        num_swdge_queues=4,
    )
    build(nc, cfg, plan, repeats)
    nc.compile()
    run = make_runner(nc, cfg.NC)
    _CACHE[key] = run
    return run


def kernel(
    x, edge_index, W1, att_src1, att_dst1, b1, W2, att_src2, att_dst2, b2, _cfg=None
):
    cfg = _cfg or Cfg()
    plan, in_maps = make_plan_and_maps(x, edge_index, cfg)
    shared = {
        "W1": np.asarray(W1, np.float32),
        "att_src1": np.asarray(att_src1, np.float32),
        "att_dst1": np.asarray(att_dst1, np.float32),
        "b1": np.asarray(b1, np.float32),
        "W2": np.asarray(W2, np.float32),
        "att_src2": np.asarray(att_src2, np.float32),
        "att_dst2": np.asarray(att_dst2, np.float32),
        "b2": np.asarray(b2, np.float32),
    }
    for m in in_maps:
        m.update(shared)
    run = _get_runner(cfg, plan)
    res = run(in_maps)
    out = np.concatenate([r["out"][: cfg.NPC] for r in res], axis=0)
    return out.astype(np.float32)


# revision 11
# speedup vs baseline: 1.4907x; 1.4907x over previous
"""Trainium2 Bass kernel for a 2-layer GAT (PyG GATConv semantics, eval mode).

v2 strategy (8 NeuronCores, SPMD, dst-sharded):
  - Core c owns dst nodes [c*NPC, (c+1)*NPC).  Phase A computes the node
    feature slab [h1 | a_src] (128 bf16 per row) plus an SBUF-resident a_dst
    table; slabs are AllGathered into a 100352-row bf16 table.
  - Edges (with self-loops) are grouped per (dst-block b, src-core chunk c)
    and sorted by local dst.  One dma_gather per (b, c) fetches the 256-B
    table rows by src; the real edge count rides in a runtime register with
    trailing -1 indices, so padding costs no DMA packets.
  - a_dst is never gathered: per (b, c) a transposed mask (node-on-partition)
    is built from host-precomputed per-node slot ranges with two DVE compare
    ops, and small TensorE matmuls broadcast a_dst to edge slots.
  - Per edge: w = exp(leaky_relu(a_src + a_dst)) (softmax shift invariance
    makes the segment max unnecessary).  S = [w | w*h] in bf16 is aggregated
    per dst block with 0/1 mask matmuls accumulating in PSUM f32.
  - Block epilogue: divide by the softmax denominator, bias, ELU, and the
    layer-2 augmented projection; AllGather slab2; run the same edge pipeline
    for layer 2 (single head) followed by log_softmax.
"""

import sys

if "/opt/trn_rl_repo" not in sys.path:
    sys.path.insert(0, "/opt/trn_rl_repo")

from dataclasses import dataclass

import numpy as np
import ml_dtypes

import concourse.bass as bass
import concourse.bacc as bacc
import concourse.tile as tile
import concourse.mybir as mybir
from concourse.masks import make_identity

F32 = mybir.dt.float32
BF16 = mybir.dt.bfloat16
I16 = mybir.dt.int16
I32 = mybir.dt.int32
BF = ml_dtypes.bfloat16

NEG_SLOPE = 0.2
SENT = 584.0  # dstv sentinel (exactly representable in bf16)


@dataclass(frozen=True)
class Cfg:
    N: int = 100000
    F: int = 128
    H1: int = 8
    C1: int = 8
    D2: int = 40
    NC: int = 8
    dbg: bool = False

    @property
    def D1(self):
        return self.H1 * self.C1  # 64

    @property
    def NPC(self):
        return self.N // self.NC  # 12500

    @property
    def NB(self):
        return (self.NPC + 127) // 128  # 98

    @property
    def NPCP(self):
        return self.NB * 128  # 12544


@dataclass(frozen=True)
class Plan:
    """Static slot structure shared by all cores (from max-over-core counts)."""
    tpbc: tuple  # [NB*NC] subtiles per (block, chunk), flattened b*NC+c
    maxsub: int  # max subtiles of any (b, c)

    @property
    def nreg(self):
        return len(self.tpbc)

    def slots(self, r):
        return self.tpbc[r] * 128

    @property
    def totsub(self):
        return sum(self.tpbc)

    @property
    def totslot(self):
        return self.totsub * 128


# ---------------------------------------------------------------- host side


NCHUNK = 4  # src chunks of 2 cores each (25088 table rows < int16 reach)


def make_plan_and_maps(x, edge_index, cfg: Cfg):
    c = cfg
    src = np.concatenate([np.asarray(edge_index[0]), np.arange(c.N)]).astype(np.int64)
    dst = np.concatenate([np.asarray(edge_index[1]), np.arange(c.N)]).astype(np.int64)
    score = src // c.NPC
    chunk = score // 2            # src chunk of two cores
    lrow = (score - chunk * 2) * c.NPCP + (src - score * c.NPC)  # row in chunk

    per_core = []
    for core in range(c.NC):
        lo = core * c.NPC
        m = (dst >= lo) & (dst < lo + c.NPC)
        s_c, s_l, d_l = chunk[m], lrow[m], dst[m] - lo
        b = d_l >> 7
        key = b * NCHUNK + s_c
        order = np.lexsort((d_l, key))
        per_core.append((key[order], s_l[order], d_l[order]))

    NREG = c.NB * NCHUNK
    cnts = np.zeros((c.NC, NREG), np.int64)
    for core in range(c.NC):
        cnts[core] = np.bincount(per_core[core][0], minlength=NREG)
    tpbc = np.maximum(1, np.ceil(cnts.max(axis=0) / 128.0)).astype(np.int64)
    plan = Plan(tpbc=tuple(int(t) for t in tpbc), maxsub=int(tpbc.max()))

    off_slot = np.zeros(NREG + 1, np.int64)
    off_slot[1:] = np.cumsum(tpbc * 128)
    off_sub = np.zeros(NREG + 1, np.int64)
    off_sub[1:] = np.cumsum(tpbc)
    TOTSLOT, TOTSUB = int(off_slot[-1]), int(off_sub[-1])

    iota_slot = np.tile(np.arange(plan.maxsub * 128, dtype=np.float32), (128, 1))
    iota_row = np.tile(np.arange(128, dtype=BF), (128, 1))
    blockmask = np.zeros((c.D1, c.H1), np.float32)
    for h in range(c.H1):
        blockmask[h * c.C1 : (h + 1) * c.C1, h] = 1.0

    in_maps = []
    for core in range(c.NC):
        key, s_l, d_l = per_core[core]
        starts = np.zeros(NREG, np.int64)
        cc = np.bincount(key, minlength=NREG)
        starts[1:] = np.cumsum(cc)[:-1]

        gidx = np.zeros(TOTSLOT, np.int16)  # padding duplicates row 0 of the chunk
        dstv = np.full(TOTSLOT, SENT, BF)
        se = np.zeros((128, NREG * 2), np.float32)
        for r in range(NREG):
            n = int(cc[r])
            st = int(starts[r])
            o = int(off_slot[r])
            if n == 0:
                continue
            gidx[o : o + n] = s_l[st : st + n]
            gidx[o + n : o + int(plan.slots(r))] = s_l[st]  # dup padding
            dl = d_l[st : st + n]
            dstv[o : o + n] = (dl & 127).astype(BF)
            # per-node slot ranges within this (b, c) for the transposed mask
            nib = (dl & 127).astype(np.int64)
            deg = np.bincount(nib, minlength=128)
            e_ = np.cumsum(deg)
            s_ = e_ - deg
            se[:, 2 * r] = s_.astype(np.float32)
            se[:, 2 * r + 1] = e_.astype(np.float32)

        # wrap gidx per region: [slots] -> [128, slots/16]
        gw = np.zeros((128, TOTSLOT // 16), np.int16)
        for r in range(NREG):
            o, s = int(off_slot[r]), int(plan.slots(r))
            w = gidx[o : o + s].reshape(s // 16, 16).T  # [16, s/16]
            gw[:, o // 16 : (o + s) // 16] = np.tile(w, (8, 1))

        dstv2 = dstv.reshape(TOTSUB, 128).T.copy()  # [128, TOTSUB]

        xs = np.zeros((c.NPCP, c.F), np.float32)
        xs[: c.NPC] = np.asarray(x)[core * c.NPC : (core + 1) * c.NPC]

        in_maps.append(
            {
                "x_slice": xs,
                "gidx": gw,
                "dstv": dstv2,
                "se": se,
                "iota_slot": iota_slot,
                "iota_row": iota_row,
                "blockmask": blockmask,
            }
        )
    return plan, in_maps


# -------------------------------------------------------------- device side


def build(nc, cfg: Cfg, plan: Plan, repeats: int = 1):
    c = cfg
    D1, D2, H1 = c.D1, c.D2, c.H1
    TROWS = c.NC * c.NPCP
    NREG = c.NB * NCHUNK
    off_slot = np.zeros(NREG + 1, np.int64)
    off_slot[1:] = np.cumsum(np.asarray(plan.tpbc) * 128)
    off_sub = np.zeros(NREG + 1, np.int64)
    off_sub[1:] = np.cumsum(np.asarray(plan.tpbc))

    x_slice = nc.dram_tensor("x_slice", [c.NPCP, c.F], F32, kind="ExternalInput")
    W1 = nc.dram_tensor("W1", [c.F, D1], F32, kind="ExternalInput")
    att_src1 = nc.dram_tensor("att_src1", [H1, c.C1], F32, kind="ExternalInput")
    att_dst1 = nc.dram_tensor("att_dst1", [H1, c.C1], F32, kind="ExternalInput")
    b1 = nc.dram_tensor("b1", [D1], F32, kind="ExternalInput")
    W2 = nc.dram_tensor("W2", [D1, D2], F32, kind="ExternalInput")
    att_src2 = nc.dram_tensor("att_src2", [1, D2], F32, kind="ExternalInput")
    att_dst2 = nc.dram_tensor("att_dst2", [1, D2], F32, kind="ExternalInput")
    b2 = nc.dram_tensor("b2", [D2], F32, kind="ExternalInput")
    gidx_t = nc.dram_tensor("gidx", [128, plan.totslot // 16], I16, kind="ExternalInput")
    dstv_t = nc.dram_tensor("dstv", [128, plan.totsub], BF16, kind="ExternalInput")
    se_t = nc.dram_tensor("se", [128, NREG * 2], F32, kind="ExternalInput")
    iota_slot_t = nc.dram_tensor("iota_slot", [128, plan.maxsub * 128], F32, kind="ExternalInput")
    iota_row_t = nc.dram_tensor("iota_row", [128, 128], BF16, kind="ExternalInput")
    bmask_t = nc.dram_tensor("blockmask", [D1, H1], F32, kind="ExternalInput")
    out_t = nc.dram_tensor("out", [c.NPCP, D2], F32, kind="ExternalOutput")

    with tile.TileContext(nc) as tc:
        with (
            tc.tile_pool(name="dram", bufs=1, space="DRAM") as dram,
            tc.tile_pool(name="cst", bufs=1) as cst,
            tc.tile_pool(name="wk", bufs=2) as wk,
            tc.tile_pool(name="gp", bufs=8) as gp,
            tc.tile_pool(name="ps", bufs=1, space="PSUM") as ps,
        ):
            # ---- constants
            ident = cst.tile([128, 128], F32)
            make_identity(nc, ident[:])
            iota = cst.tile([128, 128], BF16)
            nc.sync.dma_start(out=iota[:], in_=iota_row_t.ap())
            iosl = cst.tile([128, plan.maxsub * 128], F32)
            nc.sync.dma_start(out=iosl[:], in_=iota_slot_t.ap())
            dstv = cst.tile([128, plan.totsub], BF16)
            nc.sync.dma_start(out=dstv[:], in_=dstv_t.ap())
            se = cst.tile([128, NREG * 2], F32)
            nc.sync.dma_start(out=se[:], in_=se_t.ap())

            w1sb = cst.tile([c.F, D1], F32)
            nc.sync.dma_start(out=w1sb[:], in_=W1.ap())
            w1T_ps = ps.tile([D1, c.F], F32, tag="pT", bufs=2)
            nc.tensor.transpose(out=w1T_ps[:], in_=w1sb[:], identity=ident[:])
            w1T = cst.tile([D1, c.F], F32)
            nc.vector.tensor_copy(out=w1T[:], in_=w1T_ps[:])

            bmask = cst.tile([D1, H1], F32)
            nc.sync.dma_start(out=bmask[:], in_=bmask_t.ap())
            atts_c = cst.tile([D1, 2], F32)
            nc.sync.dma_start(
                out=atts_c[:, 0:1], in_=att_src1.ap().rearrange("h c -> (h c)")[:, None]
            )
            nc.sync.dma_start(
                out=atts_c[:, 1:2], in_=att_dst1.ap().rearrange("h c -> (h c)")[:, None]
            )
            ablk = cst.tile([D1, 2 * H1], F32)
            nc.vector.tensor_tensor(
                out=ablk[:, 0:H1],
                in0=atts_c[:, 0:1].to_broadcast([D1, H1]),
                in1=bmask[:],
                op=mybir.AluOpType.mult,
            )
            nc.vector.tensor_tensor(
                out=ablk[:, H1 : 2 * H1],
                in0=atts_c[:, 1:2].to_broadcast([D1, H1]),
                in1=bmask[:],
                op=mybir.AluOpType.mult,
            )
            acols_ps = ps.tile([c.F, 2 * H1], F32, tag="pT", bufs=2)
            nc.tensor.matmul(out=acols_ps[:], lhsT=w1T[:], rhs=ablk[:], start=True, stop=True)
            w1aug = cst.tile([c.F, D1 + 2 * H1], F32)
            nc.vector.tensor_copy(out=w1aug[:, 0:D1], in_=w1sb[:])
            nc.vector.tensor_copy(out=w1aug[:, D1 : D1 + 2 * H1], in_=acols_ps[:])

            w2sb = cst.tile([D1, D2], F32)
            nc.sync.dma_start(out=w2sb[:], in_=W2.ap())
            w2T_ps = ps.tile([D2, D1], F32, tag="pT", bufs=2)
            nc.tensor.transpose(out=w2T_ps[:], in_=w2sb[:], identity=ident[0:D1, 0:D1])
            w2T = cst.tile([D2, D1], F32)
            nc.vector.tensor_copy(out=w2T[:], in_=w2T_ps[:])
            att2 = cst.tile([D2, 2], F32)
            nc.sync.dma_start(out=att2[:, 0:1], in_=att_src2.ap().rearrange("o c -> (o c)")[:, None])
            nc.sync.dma_start(out=att2[:, 1:2], in_=att_dst2.ap().rearrange("o c -> (o c)")[:, None])
            v2_ps = ps.tile([D1, 2], F32, tag="pT", bufs=2)
            nc.tensor.matmul(out=v2_ps[:], lhsT=w2T[:], rhs=att2[:], start=True, stop=True)
            w2aug = cst.tile([D1, D2 + 2], F32)
            nc.vector.tensor_copy(out=w2aug[:, 0:D2], in_=w2sb[:])
            nc.vector.tensor_copy(out=w2aug[:, D2 : D2 + 2], in_=v2_ps[:])

            ones1 = cst.tile([1, 128], F32)
            nc.vector.memset(ones1[:], 1.0)
            b1row = cst.tile([1, D1], F32)
            nc.sync.dma_start(out=b1row[:], in_=b1.ap()[None, :])
            b1rep_ps = ps.tile([128, D1], F32, tag="pT", bufs=2)
            nc.tensor.matmul(out=b1rep_ps[:], lhsT=ones1[:], rhs=b1row[:], start=True, stop=True)
            b1rep = cst.tile([128, D1], F32)
            nc.vector.tensor_copy(out=b1rep[:], in_=b1rep_ps[:])
            b2row = cst.tile([1, D2], F32)
            nc.sync.dma_start(out=b2row[:], in_=b2.ap()[None, :])
            b2rep_ps = ps.tile([128, D2], F32, tag="pT", bufs=2)
            nc.tensor.matmul(out=b2rep_ps[:], lhsT=ones1[:], rhs=b2row[:], start=True, stop=True)
            b2rep = cst.tile([128, D2], F32)
            nc.vector.tensor_copy(out=b2rep[:], in_=b2rep_ps[:])

            # a_dst tables (SBUF resident, rewritten per rep)
            adall1 = cst.tile([128, c.NB * H1], BF16)
            adall2 = cst.tile([128, c.NB], BF16)

            for _rep in range(repeats):
                slab1 = dram.tile([c.NPCP, 128], BF16, tag=f"slab1_{_rep}")
                table1 = dram.tile([TROWS, 128], BF16, addr_space="Shared", tag=f"table1_{_rep}")
                slab2 = dram.tile([c.NPCP, 128], BF16, tag=f"slab2_{_rep}")
                table2 = dram.tile([TROWS, 128], BF16, addr_space="Shared", tag=f"table2_{_rep}")

                # ---- phase A: node features, attention columns
                for t in range(c.NB):
                    xt = wk.tile([128, c.F], F32, tag="xt")
                    nc.sync.dma_start(out=xt[:], in_=x_slice.ap()[t * 128 : (t + 1) * 128, :])
                    xT_ps = ps.tile([c.F, 128], F32, tag="pT", bufs=2)
                    nc.tensor.transpose(out=xT_ps[:], in_=xt[:], identity=ident[:])
                    xTs = wk.tile([c.F, 128], F32, tag="xTs")
                    nc.vector.tensor_copy(out=xTs[:], in_=xT_ps[:])
                    h_ps = ps.tile([128, D1 + 2 * H1], F32, tag="psA", bufs=2)
                    nc.tensor.matmul(out=h_ps[:], lhsT=xTs[:], rhs=w1aug[:], start=True, stop=True)
                    s1o = wk.tile([128, 128], BF16, tag="s1o")
                    nc.vector.memset(s1o[:, D1 + H1 : 128], 0.0)
                    nc.vector.tensor_copy(out=s1o[:, 0 : D1 + H1], in_=h_ps[:, 0 : D1 + H1])
                    nc.sync.dma_start(out=slab1[t * 128 : (t + 1) * 128, :], in_=s1o[:])
                    nc.vector.tensor_copy(
                        out=adall1[:, t * H1 : (t + 1) * H1],
                        in_=h_ps[:, D1 + H1 : D1 + 2 * H1],
                    )

                nc.gpsimd.collective_compute(
                    "AllGather",
                    mybir.AluOpType.bypass,
                    replica_groups=[list(range(c.NC))],
                    ins=[slab1.opt()],
                    outs=[table1.opt()],
                )

                maxbsub = max(
                    sum(plan.tpbc[b * NCHUNK : (b + 1) * NCHUNK]) for b in range(c.NB)
                )

                def edge_layer(layer):
                    NH = H1 if layer == 1 else 1
                    DV = D1 if layer == 1 else D2
                    SW = NH + DV
                    table = table1 if layer == 1 else table2
                    adall = adall1 if layer == 1 else adall2
                    for b in range(c.NB):
                        agg = ps.tile([128, SW], F32, tag="agg", bufs=3)
                        r0 = b * NCHUNK
                        BSUB = sum(plan.tpbc[r0 : r0 + NCHUNK])
                        # per-block slice of wrapped gather indices
                        gcols0 = int(off_slot[r0]) // 16
                        gcolsN = int(off_slot[r0 + NCHUNK]) // 16 - gcols0
                        gix = wk.tile([128, gcolsN], I16, tag="gix", bufs=3,
                                      padded_shape=[128, (maxbsub + 1) * 8])
                        nc.sync.dma_start(out=gix[:], in_=gidx_t.ap()[:, gcols0 : gcols0 + gcolsN])
                        g = gp.tile([128, maxbsub * 128], BF16, tag="g", bufs=3)
                        g3 = g[:].rearrange("p (t e) -> p t e", e=128)
                        edps = ps.tile([128, maxbsub * NH], F32, tag="psA", bufs=2)
                        boff = 0
                        for ci in range(NCHUNK):
                            r = r0 + ci
                            T = plan.tpbc[r]
                            S_r = T * 128
                            nc.gpsimd.dma_gather(
                                out_ap=g3[:, boff : boff + T, :],
                                in_ap=table[:][ci * 2 * c.NPCP : (ci + 1) * 2 * c.NPCP, :],
                                idxs_ap=gix[:, boff * 8 : boff * 8 + S_r // 16],
                                num_idxs=S_r,
                                num_idxs_reg=S_r,
                                elem_size=128,
                                single_packet=False,
                                queue_num=r % 4,
                            )
                            # transposed mask from per-node slot ranges
                            mT = wk.tile([128, plan.maxsub * 128], BF16, tag="mT", bufs=4)
                            t2 = wk.tile([128, plan.maxsub * 128], BF16, tag="t2", bufs=2)
                            nc.vector.tensor_scalar(
                                out=t2[:, 0:S_r],
                                in0=iosl[:, 0:S_r],
                                scalar1=se[:, 2 * r + 1 : 2 * r + 2],
                                scalar2=None,
                                op0=mybir.AluOpType.is_lt,
                            )
                            nc.vector.scalar_tensor_tensor(
                                out=mT[:, 0:S_r],
                                in0=iosl[:, 0:S_r],
                                scalar=se[:, 2 * r : 2 * r + 1],
                                in1=t2[:, 0:S_r],
                                op0=mybir.AluOpType.is_ge,
                                op1=mybir.AluOpType.mult,
                            )
                            # a_dst broadcast to edge slots: ed[e, h]
                            for j in range(T):
                                nc.tensor.matmul(
                                    out=edps[:, (boff + j) * NH : (boff + j + 1) * NH],
                                    lhsT=mT[:, j * 128 : (j + 1) * 128],
                                    rhs=adall[:, b * NH : (b + 1) * NH],
                                    start=True,
                                    stop=True,
                                )
                            boff += T
                        # block-batched: e = a_src + a_dst ; w = exp(lrelu(e))
                        ew = wk.tile([128, maxbsub * NH], BF16, tag="ew", bufs=3)
                        nc.scalar.copy(out=ew[:, : BSUB * NH], in_=edps[:, : BSUB * NH])
                        nc.vector.tensor_tensor(
                            out=ew[:, : BSUB * NH].rearrange("p (t h) -> p t h", h=NH),
                            in0=g3[:, 0:BSUB, DV : DV + NH],
                            in1=ew[:, : BSUB * NH].rearrange("p (t h) -> p t h", h=NH),
                            op=mybir.AluOpType.add,
                        )
                        nc.vector.scalar_tensor_tensor(
                            out=ew[:, : BSUB * NH],
                            in0=ew[:, : BSUB * NH],
                            scalar=NEG_SLOPE,
                            in1=ew[:, : BSUB * NH],
                            op0=mybir.AluOpType.mult,
                            op1=mybir.AluOpType.max,
                        )
                        nc.scalar.activation(
                            out=ew[:, : BSUB * NH], in_=ew[:, : BSUB * NH],
                            func=mybir.ActivationFunctionType.Exp,
                        )
                        # mask (edge-on-partition) and S
                        sub0 = int(off_sub[r0])
                        mask = wk.tile([128, maxbsub * 128], BF16, tag="mask", bufs=2)
                        nc.vector.tensor_tensor(
                            out=mask[:, : BSUB * 128].rearrange("p (t n) -> p t n", n=128),
                            in0=dstv[:, sub0 : sub0 + BSUB][:, :, None].to_broadcast(
                                [128, BSUB, 128]
                            ),
                            in1=iota[:][:, None, :].to_broadcast([128, BSUB, 128]),
                            op=mybir.AluOpType.is_equal,
                        )
                        S = wk.tile([128, maxbsub * SW], BF16, tag="S", bufs=2)
                        S3 = S[:, : BSUB * SW].rearrange("p (t e) -> p t e", e=SW)
                        nc.vector.tensor_copy(
                            out=S3[:, :, 0:NH],
                            in_=ew[:, : BSUB * NH].rearrange("p (t h) -> p t h", h=NH),
                        )
                        if layer == 1:
                            nc.vector.tensor_tensor(
                                out=S3[:, :, NH:SW].rearrange("p t (h ch) -> p t h ch", h=NH),
                                in0=g3[:, 0:BSUB, 0:DV].rearrange("p t (h ch) -> p t h ch", h=NH),
                                in1=ew[:, : BSUB * NH]
                                .rearrange("p (t h) -> p t h", h=NH)[:, :, :, None]
                                .to_broadcast([128, BSUB, NH, c.C1]),
                                op=mybir.AluOpType.mult,
                            )
                        else:
                            nc.vector.tensor_tensor(
                                out=S3[:, :, NH:SW],
                                in0=g3[:, 0:BSUB, 0:DV],
                                in1=ew[:, : BSUB * NH][:, :, None].to_broadcast([128, BSUB, DV]),
                                op=mybir.AluOpType.mult,
                            )
                        for j in range(BSUB):
                            nc.tensor.matmul(
                                out=agg[:],
                                lhsT=mask[:, j * 128 : (j + 1) * 128],
                                rhs=S3[:, j, :],
                                start=(j == 0),
                                stop=(j == BSUB - 1),
                            )
                            subdone += T

                        # ---- block epilogue
                        ssafe = wk.tile([128, NH], F32, tag="ssafe")
                        nc.vector.tensor_scalar_max(out=ssafe[:], in0=agg[:, 0:NH], scalar1=1e-16)
                        rec = wk.tile([128, NH], F32, tag="rec")
                        nc.vector.reciprocal(out=rec[:], in_=ssafe[:])
                        o1 = wk.tile([128, DV], F32, tag="o1")
                        if layer == 1:
                            nc.vector.tensor_tensor(
                                out=o1[:].rearrange("p (h ch) -> p h ch", h=NH),
                                in0=agg[:, NH:SW].rearrange("p (h ch) -> p h ch", h=NH),
                                in1=rec[:][:, :, None].to_broadcast([128, NH, c.C1]),
                                op=mybir.AluOpType.mult,
                            )
                            nc.vector.tensor_add(out=o1[:], in0=o1[:], in1=b1rep[:])
                            neg = wk.tile([128, DV], F32, tag="neg")
                            nc.vector.tensor_scalar_min(out=neg[:], in0=o1[:], scalar1=0.0)
                            nc.scalar.activation(
                                out=neg[:], in_=neg[:], func=mybir.ActivationFunctionType.Exp
                            )
                            pos = wk.tile([128, DV], F32, tag="pos")
                            nc.vector.tensor_scalar_max(out=pos[:], in0=o1[:], scalar1=0.0)
                            elu = wk.tile([128, DV], F32, tag="elu")
                            nc.vector.scalar_tensor_tensor(
                                out=elu[:], in0=neg[:], scalar=-1.0, in1=pos[:],
                                op0=mybir.AluOpType.add, op1=mybir.AluOpType.add,
                            )
                            eT_ps = ps.tile([DV, 128], F32, tag="pT", bufs=2)
                            nc.tensor.transpose(out=eT_ps[:], in_=elu[:], identity=ident[:])
                            eT = wk.tile([DV, 128], F32, tag="eT")
                            nc.vector.tensor_copy(out=eT[:], in_=eT_ps[:])
                            h2_ps = ps.tile([128, D2 + 2], F32, tag="psA", bufs=2)
                            nc.tensor.matmul(out=h2_ps[:], lhsT=eT[:], rhs=w2aug[:], start=True, stop=True)
                            s2o = wk.tile([128, 128], BF16, tag="s2o")
                            nc.vector.memset(s2o[:, D2 + 1 : 128], 0.0)
                            nc.vector.tensor_copy(out=s2o[:, 0 : D2 + 1], in_=h2_ps[:, 0 : D2 + 1])
                            nc.sync.dma_start(out=slab2[b * 128 : (b + 1) * 128, :], in_=s2o[:])
                            nc.vector.tensor_copy(
                                out=adall2[:, b : b + 1], in_=h2_ps[:, D2 + 1 : D2 + 2]
                            )
                        else:
                            nc.vector.tensor_tensor(
                                out=o1[:],
                                in0=agg[:, NH:SW],
                                in1=rec[:].to_broadcast([128, DV]),
                                op=mybir.AluOpType.mult,
                            )
                            nc.vector.tensor_add(out=o1[:], in0=o1[:], in1=b2rep[:])
                            mx = wk.tile([128, 1], F32, tag="mx")
                            nc.vector.tensor_reduce(
                                out=mx[:], in_=o1[:], axis=mybir.AxisListType.X,
                                op=mybir.AluOpType.max,
                            )
                            xm = wk.tile([128, DV], F32, tag="xm")
                            nc.vector.tensor_tensor(
                                out=xm[:], in0=o1[:], in1=mx[:].to_broadcast([128, DV]),
                                op=mybir.AluOpType.subtract,
                            )
                            ex = wk.tile([128, DV], F32, tag="ex")
                            sm = wk.tile([128, 1], F32, tag="sm")
                            nc.scalar.activation(
                                out=ex[:], in_=xm[:], func=mybir.ActivationFunctionType.Exp
                            )
                            nc.vector.tensor_reduce(
                                out=sm[:], in_=ex[:], axis=mybir.AxisListType.X,
                                op=mybir.AluOpType.add,
                            )
                            lg = wk.tile([128, 1], F32, tag="lg")
                            nc.scalar.activation(
                                out=lg[:], in_=sm[:], func=mybir.ActivationFunctionType.Ln
                            )
                            oo = wk.tile([128, DV], F32, tag="oo")
                            nc.vector.tensor_tensor(
                                out=oo[:], in0=xm[:], in1=lg[:].to_broadcast([128, DV]),
                                op=mybir.AluOpType.subtract,
                            )
                            nc.sync.dma_start(out=out_t.ap()[b * 128 : (b + 1) * 128, :], in_=oo[:])

                edge_layer(1)
                nc.gpsimd.collective_compute(
                    "AllGather",
                    mybir.AluOpType.bypass,
                    replica_groups=[list(range(c.NC))],
                    ins=[slab2.opt()],
                    outs=[table2.opt()],
                )
                edge_layer(2)


# ------------------------------------------------------------------ driver


def make_runner(nc, n_cores=8):
    import jax
    from jax.sharding import Mesh, PartitionSpec
    from jax.experimental.shard_map import shard_map
    from concourse.bass2jax import (
        _bass_exec_p,
        install_neuronx_cc_hook,
        partition_id_tensor,
    )

    install_neuronx_cc_hook()
    partition_name = nc.partition_id_tensor.name if nc.partition_id_tensor else None

    in_names, out_names, out_avals, zero_outs = [], [], [], []
    for alloc in nc.m.functions[0].allocations:
        if not isinstance(alloc, mybir.MemoryLocationSet):
            continue
        name = alloc.memorylocations[0].name
        if alloc.kind == "ExternalInput":
            if name != partition_name:
                in_names.append(name)
        elif alloc.kind == "ExternalOutput":
            shape = tuple(alloc.tensor_shape)
            dtype = mybir.dt.np(alloc.dtype)
            out_names.append(name)
            out_avals.append(jax.core.ShapedArray(shape, dtype))
            zero_outs.append(np.zeros(shape, dtype))
    n_params = len(in_names)
    n_outs = len(out_avals)
    all_in_names = list(in_names) + list(out_names)
    if partition_name is not None:
        all_in_names.append(partition_name)

    donate = tuple(range(n_params, n_params + n_outs))

    def _body(*args):
        operands = list(args)
        if partition_name is not None:
            operands.append(partition_id_tensor())
        outs = _bass_exec_p.bind(
            *operands,
            out_avals=tuple(out_avals),
            in_names=tuple(all_in_names),
            out_names=tuple(out_names),
            lowering_input_output_aliases=(),
            sim_require_finite=False,
            sim_require_nnan=False,
            nc=nc,
        )
        return tuple(outs)

    devices = jax.devices()[:n_cores]
    mesh = Mesh(np.asarray(devices), ("core",))
    in_specs = (PartitionSpec("core"),) * (n_params + n_outs)
    out_specs = (PartitionSpec("core"),) * len(out_names)
    sharded = jax.jit(
        shard_map(_body, mesh=mesh, in_specs=in_specs, out_specs=out_specs, check_rep=False),
        donate_argnums=donate,
        keep_unused=True,
    )

    def run(in_maps):
        per_core = [[np.asarray(m[name]) for name in in_names] for m in in_maps]
        concat_in = [
            np.concatenate([per_core[cc][i] for cc in range(n_cores)], axis=0)
            for i in range(n_params)
        ]
        concat_zeros = [
            np.zeros((n_cores * z.shape[0], *z.shape[1:]), z.dtype) for z in zero_outs
        ]
        out_arrs = sharded(*concat_in, *concat_zeros)
        jax.block_until_ready(out_arrs)
        return [
            {
                name: np.asarray(out_arrs[i]).reshape(n_cores, *out_avals[i].shape)[cc]
                for i, name in enumerate(out_names)
            }
            for cc in range(n_cores)
        ]

    return run


_CACHE = {}


def _get_runner(cfg: Cfg, plan: Plan, repeats: int = 1):
    key = (cfg, plan, repeats)
    if key in _CACHE:
        return _CACHE[key]
    nc = bacc.Bacc(
        "TRN2",
        target_bir_lowering=False,
        debug=False,
        num_devices=cfg.NC,
        num_swdge_queues=4,
    )
    build(nc, cfg, plan, repeats)
    nc.compile()
    run = make_runner(nc, cfg.NC)
    _CACHE[key] = run
    return run


def kernel(
    x, edge_index, W1, att_src1, att_dst1, b1, W2, att_src2, att_dst2, b2, _cfg=None
):
    cfg = _cfg or Cfg()
    plan, in_maps = make_plan_and_maps(x, edge_index, cfg)
    shared = {
        "W1": np.asarray(W1, np.float32),
        "att_src1": np.asarray(att_src1, np.float32),
        "att_dst1": np.asarray(att_dst1, np.float32),
        "b1": np.asarray(b1, np.float32),
        "W2": np.asarray(W2, np.float32),
        "att_src2": np.asarray(att_src2, np.float32),
        "att_dst2": np.asarray(att_dst2, np.float32),
        "b2": np.asarray(b2, np.float32),
    }
    for m in in_maps:
        m.update(shared)
    run = _get_runner(cfg, plan)
    res = run(in_maps)
    out = np.concatenate([r["out"][: cfg.NPC] for r in res], axis=0)
    return out.astype(np.float32)
